# revision 3
# baseline (speedup 1.0000x reference)
"""Dynamic filter layer on 8 trn2 NeuronCores — v5 (bf16, DVE+Pool split,
bf16 shifted-identity PE accumulate).

out[b,i,j,c] = sum_{di,dj} x[b,i+di,j+dj,c] * flow[b,i,j,di*K+dj]

B=8, H=W=256, C=64, K=5, Ho=Wo=252. Sharding: data-parallel over batch,
one sample per core (SPMD, no collectives).

v5 design (per core), HW-microbench-calibrated:
  - measured on HW: DVE tensor_tensor [128,2048] bf16 ~2.3us (1x only; the
    modeled 2x_1p perf mode does NOT engage, and broadcast APs are
    slightly FASTER than packed reads). So: plain stride-0 channel
    broadcast of the flow value, no dup trick, no bf16-vs-f32 DVE gain.
  - bf16 still pays for PE (1 cycle/row vs 4 for fp32) and halves DMA.
  - no tap pre-adds: each of the 25 taps is one product tensor
    (11 on DVE / 14 on Pool, interleaved) which the PE immediately
    shift-accumulates into PSUM (25 streams x 8 bank-matmuls N=512).
  - column chunks of 64: psum tile [124,64,64] f32 = all 8 banks
    (bufs=1); ACT's psum->sbuf bf16 copy hides in PE slack.
  - output staged bf16, host upcasts to f32.
Final 4 output rows (252 = 2*124 + 4) use the transposed scheme
(partition = output column, dj via 5 x copies, di on the free axis).
"""

import numpy as np

H = 256
W = 256
C = 64
K = 5
HO = H - K + 1  # 252
WO = W - K + 1  # 252
NCORES = 8
JW = 64  # column chunk width; psum tile [124, JW, C] f32 = 8 PSUM banks
BANK_J = 8  # 8 cols x 64 ch = 512 f32 = one PSUM bank

# Per-di engine pattern over dj: 'D' = DVE, 'P' = Pool. 17 DVE / 8 Pool
# (HW: DVE ~1.15 ns/elem vs Pool ~2.59), interleaved so both engines
# stay fed and the PE consumes alternately.
ENG_PATTERN = {
    0: "DPDDP",
    1: "DDPDD",
    2: "PDDPD",
    3: "DDPDD",
    4: "PDDPD",
}
N_DVE = sum(p.count("D") for p in ENG_PATTERN.values())  # 17

_nc_cache = {}


def _build(reps=1):
    """reps>1 wraps the whole body in a HW loop (timing calibration only)."""
    global _nc_cache
    if reps in _nc_cache:
        return _nc_cache[reps]

    import contextlib

    import concourse.bacc as bacc
    import concourse.tile as tile
    from concourse import mybir
    from concourse.masks import make_identity

    f32 = mybir.dt.float32
    bf16 = mybir.dt.bfloat16
    mult = mybir.AluOpType.mult

    nc = bacc.Bacc(None, target_bir_lowering=False)
    x = nc.dram_tensor("x", [H, W, C], bf16, kind="ExternalInput")
    fl = nc.dram_tensor("fl", [HO, WO, K * K], bf16, kind="ExternalInput")
    out = nc.dram_tensor("out", [HO, WO, C], bf16, kind="ExternalOutput")

    with tile.TileContext(nc) as tc:
        with (
            tc.tile_pool(name="cst", bufs=1) as cst,
            tc.tile_pool(name="xp", bufs=2) as xp,
            tc.tile_pool(name="fp", bufs=2) as fp,
            tc.tile_pool(name="td", bufs=1) as td,
            tc.tile_pool(name="sp", bufs=2) as sp,
            tc.tile_pool(name="pp", bufs=1, space="PSUM") as pp,
        ):
            ident = cst.tile([128, 128], bf16, tag="ident")
            make_identity(nc, ident)

            with tc.For_i(0, reps, 1) if reps > 1 else contextlib.nullcontext():
                # --- main blocks: out rows [0,124) and [124,248) ---
                for i0 in (0, 124):
                    for j0 in range(0, WO, JW):
                        jw = min(JW, WO - j0)
                        xw = min(jw + K - 1, W - j0)
                        xt = xp.tile([128, JW + K - 1, C], bf16, tag="x")
                        nc.sync.dma_start(
                            out=xt[:, :xw, :],
                            in_=x[i0 : i0 + 128, j0 : j0 + xw, :],
                        )
                        # fc5[di][k] = fl[i0 + k - di] (taps di*K..di*K+4);
                        # rows k < di of the top block are zeroed.
                        fc5 = []
                        for di in range(K):
                            ft = fp.tile([128, JW, K], bf16, tag=f"f{di}")
                            lo = i0 - di
                            ts0 = K * di
                            if lo >= 0:
                                nc.sync.dma_start(
                                    out=ft[:, :jw, :],
                                    in_=fl[
                                        lo : lo + 128, j0 : j0 + jw,
                                        ts0 : ts0 + K,
                                    ],
                                )
                            else:
                                nc.gpsimd.memset(ft[: -lo, :jw, :], 0.0)
                                nc.sync.dma_start(
                                    out=ft[-lo:, :jw, :],
                                    in_=fl[
                                        0 : 128 + lo, j0 : j0 + jw,
                                        ts0 : ts0 + K,
                                    ],
                                )
                            fc5.append(ft)

                        ps = pp.tile([124, JW, C], f32, tag="ps")
                        for t in range(K * K):
                            di, dj = divmod(t, K)
                            eng = (
                                nc.vector
                                if ENG_PATTERN[di][dj] == "D"
                                else nc.gpsimd
                            )
                            tag = "gd" if ENG_PATTERN[di][dj] == "D" else "gp"
                            g = td.tile([128, JW, C], bf16, tag=tag, bufs=3)
                            fbc = fc5[di][:, :jw, dj : dj + 1].to_broadcast(
                                [128, jw, C]
                            )
                            eng.tensor_tensor(
                                out=g[:, :jw, :],
                                in0=xt[:, dj : dj + jw, :],
                                in1=fbc,
                                op=mult,
                            )
                            for jj in range(0, jw, BANK_J):
                                njw = min(BANK_J, jw - jj)
                                nc.tensor.matmul(
                                    ps[:, jj : jj + njw, :],
                                    ident[:, di : di + 124],
                                    g[:, jj : jj + njw, :],
                                    start=(t == 0),
                                    stop=(t == K * K - 1),
                                )
                        stage = sp.tile([124, JW, C], bf16, tag="stage")
                        nc.scalar.copy(out=stage[:, :jw, :], in_=ps[:, :jw, :])
                        nc.sync.dma_start(
                            out=out[i0 : i0 + 124, j0 : j0 + jw, :],
                            in_=stage[:, :jw, :],
                        )

                # --- strip: out rows [248,252), transposed (partition=j) ---
                for j0, P in ((0, 124), (124, 124), (248, 4)):
                    xs5 = []
                    for dj in range(K):
                        xs = fp.tile([P, 8, C], bf16, tag=f"sx{dj}")
                        nc.sync.dma_start(
                            out=xs,
                            in_=x[
                                HO - 4 : HO + 4, j0 + dj : j0 + dj + P, :
                            ].rearrange("r j c -> j r c"),
                        )
                        xs5.append(xs)
                    fs = fp.tile([P, 4, K * K], bf16, tag="sf")
                    nc.sync.dma_start(
                        out=fs,
                        in_=fl[HO - 4 : HO, j0 : j0 + P, :].rearrange(
                            "i j t -> j i t"
                        ),
                    )
                    ps_s = pp.tile([P, 4, C], f32, tag="ps")
                    for t in range(K * K):
                        di, dj = divmod(t, K)
                        eng = nc.vector if ENG_PATTERN[di][dj] == "D" else nc.gpsimd
                        tmp = td.tile([P, 4, C], bf16, tag="st", bufs=4)
                        fbs = fs[:, :, t : t + 1].to_broadcast([P, 4, C])
                        eng.tensor_tensor(
                            out=tmp,
                            in0=xs5[dj][:, di : di + 4, :],
                            in1=fbs,
                            op=mult,
                        )
                        nc.tensor.matmul(
                            ps_s[:, :, :],
                            ident[:P, :P],
                            tmp[:, :, :],
                            start=(t == 0),
                            stop=(t == K * K - 1),
                        )
                    stage = sp.tile([P, 4, C], bf16, tag="sstage")
                    nc.scalar.copy(out=stage, in_=ps_s)
                    nc.sync.dma_start(
                        out=out[HO - 4 : HO, j0 : j0 + P, :].rearrange(
                            "i j c -> j i c"
                        ),
                        in_=stage,
                    )

    nc.finalize()
    _nc_cache[reps] = nc
    return nc


def _to_bf16(a):
    import ml_dtypes

    return np.ascontiguousarray(np.asarray(a).astype(ml_dtypes.bfloat16))


def _core_inputs(x_core, flow_core):
    """f32 [H,W,C] and [HO,WO,25] -> bf16 input map for one core."""
    return {"x": _to_bf16(x_core), "fl": _to_bf16(flow_core)}


def _postprocess_core(out_core):
    return np.asarray(out_core, dtype=np.float32)


def _run(x, flow, trace=False):
    """x: [8,H,W,C] f32, flow: [8,HO,WO,25] f32 -> (out [8,HO,WO,C], res)"""
    from concourse.bass_utils import run_bass_kernel_spmd

    nc = _build()
    in_maps = [_core_inputs(x[b], flow[b]) for b in range(NCORES)]
    res = run_bass_kernel_spmd(
        nc, in_maps, core_ids=list(range(NCORES)), trace=trace
    )
    out = np.stack(
        [_postprocess_core(r["out"]) for r in res.results], axis=0
    )
    return out, res


def kernel(x, flow, ksize=None, **_unused):
    x = np.asarray(x, dtype=np.float32)
    flow = np.asarray(flow, dtype=np.float32)
    out, _ = _run(x, flow, trace=False)
    return out
